# revision 23
# baseline (speedup 1.0000x reference)
"""Causal self-attention (B=4, T=2048, C=1024, H=16, D=64) on 8 trn2 NeuronCores.

Sharding: core c = (batch b=c//2, head-group hg=c%2 of 8 heads / 512 channels).
Each core computes attention for its 8 heads on its batch plus the partial
output projection over its 512 channels of Wp; the host sums the two partial
projections per batch and adds bp.

Per-core layout is feature-major ("transposed"): x is sent as xT (C, T) so
q/k project directly as qT = Wq.T @ x.T with both operands k(partition)-major.
v is computed in natural (T, D) orientation with a ones-column appended per
head (65-wide blocks) so the yT = [v|1].T @ P^T matmul also yields softmax
row sums.  Matmul operands are bf16; accumulation, softmax internals and the
final normalization stay fp32.

v2 schedule (vs the v1 filler-slot pipeline):
 - PE warm-up matmuls + a dummy exp at t=0 keep the HAM clock-gate window
   busy during the initial x/w DMA so phase 0 runs at 2.4 GHz and the ACT
   exp table set is resident before the first real softmax.
 - Phase 0 computes only what attention needs immediately: v for t-tiles
   0-3, the full q projection of pair 0, and the m=0 quarter of its k
   projection.  The remaining v t-groups and k quarters become gated filler
   work inside pair-0 attention, shrinking the serial projection phase from
   ~44us to ~DMA-bound ~18us.
 - S tiles are built in [128,1024] 2-bank PSUM tiles (two 512-col matmuls)
   so each softmax exp is a single 1024-wide ACTIVATE: ACT pipeline-fill
   overhead (352 cyc/instr) drops ~40%.
 - P^T rows are exp'd into a per-head SBUF slab; the P@V accumulation for a
   512-query chunk runs as one back-to-back matmul run (deferred-chunk) a
   row after the chunk's last key-tile closes, interleaved one-row early
   into the next head's S sections at head boundaries so ACT never idles.
 - kT is stored once per pair; S contracts 64 partitions directly
   (tile_position row base 0/64) instead of duplicating kT with zero halves.
 - Fillers (deferred v groups, deferred k quarters, next pair's q/k
   projections, the output projection) are sprinkled one PE instruction per
   S/chunk matmul with explicit flush gates where attention first consumes
   their outputs.
"""

import math
from collections import deque

import numpy as np

B, T, C = 4, 2048, 1024
H, D = 16, 64
NCORES = 8
PAIRS = 4          # head pairs per core (2 heads = 128 channels each)
KT = C // 128      # 8 k-tiles over input channels
MT = T // 128      # 16 tiles over sequence
SC = 1.0 / math.sqrt(D)

_CACHE = {}


def _build_nc():
    from contextlib import ExitStack

    import concourse.bacc as bacc
    import concourse.mybir as mybir
    import concourse.tile as tile

    f32 = mybir.dt.float32
    bf16 = mybir.dt.bfloat16
    AF = mybir.ActivationFunctionType

    nc = bacc.Bacc("TRN2", target_bir_lowering=False, debug=False)

    xT = nc.dram_tensor("xT", (C, T), bf16, kind="ExternalInput").ap()
    wqD = nc.dram_tensor("wq", (C, 512), bf16, kind="ExternalInput").ap()
    wkD = nc.dram_tensor("wk", (C, 512), bf16, kind="ExternalInput").ap()
    wvD = nc.dram_tensor("wv", (C, 512), bf16, kind="ExternalInput").ap()
    wpD = nc.dram_tensor("wp", (512, C), bf16, kind="ExternalInput").ap()
    bqD = nc.dram_tensor("bq", (512,), f32, kind="ExternalInput").ap()
    bkD = nc.dram_tensor("bk", (512,), f32, kind="ExternalInput").ap()
    bvD = nc.dram_tensor("bv", (512,), f32, kind="ExternalInput").ap()
    # partial projections leave the core in bf16: halves the 8MB writeback;
    # the host sums the two per-batch partials in fp32.
    outD = nc.dram_tensor("out", (T, C), bf16, kind="ExternalOutput").ap()

    # P^T slab column offsets: region j holds keys-tile j's probabilities for
    # queries 128j..2047 (width T-128j).
    OFF = []
    o = 0
    for j in range(MT):
        OFF.append(o)
        o += T - 128 * j
    SLAB = o  # 17408 cols bf16 = 34.8KB/partition

    with tile.TileContext(nc) as tc, ExitStack() as ctx:
        const = ctx.enter_context(tc.tile_pool(name="const", bufs=1))
        xp = ctx.enter_context(tc.tile_pool(name="xp", bufs=1))

        wv_sb = const.tile([128, KT, 512], bf16)
        wq_sb = const.tile([128, KT, 512], bf16)
        wk_sb = const.tile([128, KT, 512], bf16)
        wp_sb = const.tile([128, 4, C], bf16)
        xsb = [xp.tile([128, T], bf16, name=f"xsb{k}") for k in range(KT)]
        bq_sb = const.tile([128, PAIRS], f32)
        bk_sb = const.tile([128, PAIRS], f32)
        bv_row = const.tile([1, 512], f32)
        bv_bc = const.tile([128, 512], f32)
        wz = const.tile([128, 512], bf16)     # warm-up zeros
        wdum = const.tile([128, 8], bf16)     # dummy exp target
        mask_tri = const.tile([128, 128], bf16)
        v_all = const.tile([128, MT * 8 * 65], bf16)
        v4 = v_all.rearrange("p (t h e) -> p t h e", t=MT, h=8)
        qT_t = [const.tile([128, T], bf16, name=f"qT{p}") for p in range(PAIRS)]
        kT_t = [const.tile([128, T], bf16, name=f"kT{p}") for p in range(PAIRS)]
        yT_tiles = [const.tile([128, T], bf16, name=f"yT{i}") for i in range(PAIRS)]
        pt_slab = const.tile([128, SLAB], bf16)

        # ---- warm-up: keep the PE busy from t~0 so the HAM clock gate is at
        # 8/8 before the first real matmul, and preload the exp table set.
        wps = ctx.enter_context(tc.tile_pool(name="wps", bufs=1, space="PSUM"))
        wt = wps.tile([128, 512], f32, name="warm")
        nc.vector.memset(wz[:], 0.0)
        nc.scalar.activation(wdum[:], wz[:, 0:8], AF.Exp, scale=1.0)
        for _ in range(16):
            nc.tensor.matmul(wt[:], lhsT=wz[:, 0:128], rhs=wz[:], start=True,
                             stop=True)

        def warm_mm(n=1):
            # Dummy self-loading matmuls into the scratch PSUM bank: ~160ns
            # of PE-array activity each, no data deps.  Sprinkled into spots
            # where the PE would briefly starve so the HAM activity monitor
            # never re-throttles the clock to 4/8 (cumulative idle in its
            # 3.4us window triggers that).  NOTE: standalone ldweights is NOT
            # safe here - walrus pairs it with the next matmul, which then
            # consumes the dummy weights.
            for _ in range(n):
                nc.tensor.matmul(wt[:, 0:128], lhsT=wz[:, 0:128],
                                 rhs=wz[:, 0:128], start=True, stop=True)

        # mask + ones column (no DMA deps; runs immediately)
        nc.gpsimd.memset(mask_tri[:], 1.0)
        nc.gpsimd.affine_select(
            out=mask_tri[:],
            in_=mask_tri[:],
            compare_op=mybir.AluOpType.is_ge,
            fill=0.0,
            base=0,
            pattern=[[1, 128]],
            channel_multiplier=-1,
        )
        nc.gpsimd.memset(v4[:, :, :, 64:65], 1.0)

        # ---- DMA, ordered by first need; alternate the two HWDGE rings.
        nc.sync.dma_start(bq_sb[:], bqD.rearrange("(a p) -> p a", p=128))
        nc.scalar.dma_start(bk_sb[:], bkD.rearrange("(a p) -> p a", p=128))
        nc.sync.dma_start(bv_row[:], bvD.rearrange("(a n) -> a n", a=1))
        nc.gpsimd.partition_broadcast(bv_bc[:], bv_row[:])
        wv4 = wvD.rearrange("(k p) n -> p k n", p=128)
        wq4 = wqD.rearrange("(k p) n -> p k n", p=128)
        wk4 = wkD.rearrange("(k p) n -> p k n", p=128)
        wp4 = wpD.rearrange("(k p) n -> p k n", p=128)
        # phase-0 V-A needs wv + x cols 0:512
        for k in range(KT):
            ea = nc.sync if k % 2 == 0 else nc.scalar
            eb = nc.scalar if k % 2 == 0 else nc.sync
            eb.dma_start(wv_sb[:, k, :], wv4[:, k, :])
            ea.dma_start(xsb[k][:, 0:512], xT[k * 128:(k + 1) * 128, 0:512])
        # QK0-q needs wq + full x
        nc.scalar.dma_start(wq_sb[:, 0:4, :], wq4[:, 0:4, :])
        nc.sync.dma_start(wq_sb[:, 4:8, :], wq4[:, 4:8, :])
        for k in range(KT):
            ea = nc.sync if k % 2 == 0 else nc.scalar
            ea.dma_start(xsb[k][:, 512:1024],
                         xT[k * 128:(k + 1) * 128, 512:1024])
        for k in range(KT):
            eb = nc.scalar if k % 2 == 0 else nc.sync
            eb.dma_start(xsb[k][:, 1024:2048],
                         xT[k * 128:(k + 1) * 128, 1024:2048])
        nc.scalar.dma_start(wk_sb[:, 0:4, :], wk4[:, 0:4, :])
        nc.sync.dma_start(wk_sb[:, 4:8, :], wk4[:, 4:8, :])
        nc.scalar.dma_start(wp_sb[:, 0:2, :], wp4[:, 0:2, :])
        nc.sync.dma_start(wp_sb[:, 2:4, :], wp4[:, 2:4, :])

        # ---- phase 0: V t-tiles 0-3 (k-outer, DMA-paced), q proj of pair 0,
        # k proj quarter m=0.  Everything else defers into attention fillers.
        gpsum = tc.alloc_tile_pool(name="gpsum", bufs=7, space="PSUM")

        psA = [gpsum.tile([128, 512], f32, tag="gp", name=f"vA{t}")
               for t in range(4)]
        for k in range(KT):
            for t in range(4):
                nc.tensor.matmul(
                    psA[t][:],
                    lhsT=xsb[k][:, t * 128:(t + 1) * 128],
                    rhs=wv_sb[:, k, :],
                    start=(k == 0),
                    stop=(k == KT - 1),
                )
            warm_mm(2)
        for t in range(4):
            nc.vector.tensor_add(
                v4[:, t, :, 0:64],
                psA[t].rearrange("p (h e) -> p h e", h=8),
                bv_bc.rearrange("p (h e) -> p h e", h=8),
            )

        def qk0_group(qk, m):
            ms = slice(m * 512, (m + 1) * 512)
            w_sb = wq_sb if qk == 0 else wk_sb
            ps = gpsum.tile([128, 512], f32, tag="gp", name=f"qk0_{qk}_{m}")
            for k in range(KT):
                nc.tensor.matmul(
                    ps[:],
                    lhsT=w_sb[:, k, 0:128],
                    rhs=xsb[k][:, ms],
                    start=(k == 0),
                    stop=(k == KT - 1),
                )
            if qk == 0:
                nc.vector.tensor_scalar_add(qT_t[0][:, ms], ps[:], bq_sb[:, 0:1])
            else:
                # ACT is otherwise idle in phase 0
                nc.scalar.activation(kT_t[0][:, ms], ps[:], AF.Identity,
                                     bias=bk_sb[:, 0:1])

        for m in range(4):
            qk0_group(0, m)
            warm_mm(8)
        qk0_group(1, 0)
        warm_mm(8)
        gpsum.release()

        # ---- attention pools: sps 4 banks + yps 2 + qkp 2 = 8 PSUM banks.
        sps = ctx.enter_context(tc.tile_pool(name="sps", bufs=2, space="PSUM"))
        yps = ctx.enter_context(tc.tile_pool(name="yps", bufs=1, space="PSUM"))
        qkp = ctx.enter_context(tc.tile_pool(name="qkp", bufs=2, space="PSUM"))
        nrm = ctx.enter_context(tc.tile_pool(name="nrm", bufs=2))
        ostp = ctx.enter_context(tc.tile_pool(name="ost", bufs=2))
        # staged output-projection partials for the split chunks (ic 1-3):
        # up to 24 live at once between projA (head-6 fillers) and projB.
        ppart = ctx.enter_context(tc.tile_pool(name="ppart", bufs=24))

        fill_iters = deque()   # of (token, generator)
        done_toks = set()
        pace = {"n": 1, "tick": 0}

        def sprinkle():
            # pace["n"] > 1 consumes a real filler only every n-th call so a
            # pair's filler supply lasts its whole span; off-ticks emit a
            # dummy weight-load to keep the PE activity monitor warm.
            pace["tick"] += 1
            if pace["tick"] % pace["n"] != 0:
                warm_mm(1)
                return
            while fill_iters:
                tok, g = fill_iters[0]
                th = next(g, None)
                if th is None:
                    done_toks.add(tok)
                    fill_iters.popleft()
                    continue
                th()
                return
            warm_mm(2)   # dry: keep the clock gate open

        def flush(tok):
            while tok not in done_toks and fill_iters:
                t0, g = fill_iters[0]
                th = next(g, None)
                if th is None:
                    done_toks.add(t0)
                    fill_iters.popleft()
                    continue
                th()
            done_toks.add(tok)  # tolerate flushing a token never enqueued

        def drain():
            while fill_iters:
                sprinkle()

        def g_v(group):
            """Deferred v for t-tiles 4g..4g+3 (t-outer, one qkp bank)."""
            for t in range(4 * group, 4 * group + 4):
                ps = qkp.tile([128, 512], f32, tag="qk", name=f"v{t}")
                for k in range(KT):
                    def mm(ps=ps, k=k, t=t):
                        nc.tensor.matmul(
                            ps[:],
                            lhsT=xsb[k][:, t * 128:(t + 1) * 128],
                            rhs=wv_sb[:, k, :],
                            start=(k == 0),
                            stop=(k == KT - 1),
                        )
                    yield mm

                def ev(ps=ps, t=t):
                    nc.vector.tensor_add(
                        v4[:, t, :, 0:64],
                        ps.rearrange("p (h e) -> p h e", h=8),
                        bv_bc.rearrange("p (h e) -> p h e", h=8),
                    )
                yield ev

        def g_qk0k(m):
            """Deferred pair-0 k projection quarter m."""
            ms = slice(m * 512, (m + 1) * 512)
            ps = qkp.tile([128, 512], f32, tag="qk", name=f"qk0k{m}")
            for k in range(KT):
                def mm(ps=ps, k=k, ms=ms):
                    nc.tensor.matmul(
                        ps[:],
                        lhsT=wk_sb[:, k, 0:128],
                        rhs=xsb[k][:, ms],
                        start=(k == 0),
                        stop=(k == KT - 1),
                    )
                yield mm

            def ev(ps=ps, ms=ms):
                nc.vector.tensor_scalar_add(kT_t[0][:, ms], ps[:], bk_sb[:, 0:1])
            yield ev

        def g_qk(p):
            """q/k projections for pair p (fillers during pair p-1)."""
            for qk in range(2):
                w_sb = wq_sb if qk == 0 else wk_sb
                dstT = qT_t[p] if qk == 0 else kT_t[p]
                bias = bq_sb if qk == 0 else bk_sb
                for m in range(4):
                    ps = qkp.tile([128, 512], f32, tag="qk",
                                  name=f"qk{p}_{qk}_{m}")
                    for k in range(KT):
                        def mm(ps=ps, k=k, m=m, w_sb=w_sb):
                            nc.tensor.matmul(
                                ps[:],
                                lhsT=w_sb[:, k, p * 128:(p + 1) * 128],
                                rhs=xsb[k][:, m * 512:(m + 1) * 512],
                                start=(k == 0),
                                stop=(k == KT - 1),
                            )
                        yield mm

                    def ev(ps=ps, m=m, dstT=dstT, bias=bias):
                        nc.vector.tensor_scalar_add(
                            dstT[:, m * 512:(m + 1) * 512], ps[:],
                            bias[:, p:p + 1])
                    yield ev

        # Output projection: chunk 0's projection runs whole (gated on head
        # 7's chunk 0).  Chunks 1-3 are split: part A (pairs 0-2 plus pair-3
        # head 6, 4 matmuls) runs as filler during pair-3 head 6 and stages
        # a bf16 partial; part B adds head 7's contribution and evicts,
        # gated only on head 7's chunks.  This gives pair-3's first head
        # real filler work and shrinks the serial drain tail to part B of
        # chunk 3 alone.
        part_sb = {}   # (mt, oh) -> staged partial tile

        def g_proj0():
            for mt in range(4):
                ost = ostp.tile([128, C], bf16, tag="ost", name=f"ost{mt}")
                for oh in range(2):
                    pps = qkp.tile([128, 512], f32, tag="qk",
                                   name=f"pp{mt}_{oh}")
                    for k in range(4):
                        def mm(pps=pps, k=k, mt=mt, oh=oh):
                            nc.tensor.matmul(
                                pps[:],
                                lhsT=yT_tiles[k][:, mt * 128:(mt + 1) * 128],
                                rhs=wp_sb[:, k, oh * 512:(oh + 1) * 512],
                                start=(k == 0),
                                stop=(k == 3),
                            )
                        yield mm

                    def ev(pps=pps, ost=ost, mt=mt, oh=oh):
                        if oh == 0:
                            nc.vector.tensor_copy(ost[:, 0:512], pps[:])
                        else:
                            nc.scalar.activation(ost[:, 512:1024], pps[:],
                                                 AF.Copy)
                        (nc.sync if (mt * 2 + oh) % 2 == 0
                         else nc.scalar).dma_start(
                            outD[mt * 128:(mt + 1) * 128,
                                 oh * 512:(oh + 1) * 512],
                            ost[:, oh * 512:(oh + 1) * 512],
                        )
                    yield ev

        def g_projA(ic):
            for mt in range(4 * ic, 4 * ic + 4):
                for oh in range(2):
                    pps = qkp.tile([128, 512], f32, tag="qk",
                                   name=f"pA{mt}_{oh}")
                    for k in range(4):
                        def mm(pps=pps, k=k, mt=mt, oh=oh):
                            if k < 3:
                                nc.tensor.matmul(
                                    pps[:],
                                    lhsT=yT_tiles[k][:, mt * 128:(mt + 1) * 128],
                                    rhs=wp_sb[:, k, oh * 512:(oh + 1) * 512],
                                    start=(k == 0),
                                    stop=False,
                                )
                            else:
                                nc.tensor.matmul(
                                    pps[:],
                                    lhsT=yT_tiles[3][0:64,
                                                     mt * 128:(mt + 1) * 128],
                                    rhs=wp_sb[0:64, 3, oh * 512:(oh + 1) * 512],
                                    start=False,
                                    stop=True,
                                )
                        yield mm

                    def ev(pps=pps, mt=mt, oh=oh):
                        pt = ppart.tile([128, 512], bf16, tag="pp",
                                        name=f"ps{mt}_{oh}")
                        part_sb[(mt, oh)] = pt
                        nc.vector.tensor_copy(pt[:], pps[:])
                    yield ev

        def g_projB(ic):
            for mt in range(4 * ic, 4 * ic + 4):
                ost = ostp.tile([128, C], bf16, tag="ost", name=f"ost{mt}")
                for oh in range(2):
                    pps = qkp.tile([128, 512], f32, tag="qk",
                                   name=f"pB{mt}_{oh}")

                    def mm(pps=pps, mt=mt, oh=oh):
                        nc.tensor.matmul(
                            pps[:],
                            lhsT=yT_tiles[3][64:128, mt * 128:(mt + 1) * 128],
                            rhs=wp_sb[64:128, 3, oh * 512:(oh + 1) * 512],
                            start=True,
                            stop=True,
                        )
                    yield mm

                    def ev(pps=pps, ost=ost, mt=mt, oh=oh):
                        nc.vector.tensor_add(
                            ost[:, oh * 512:(oh + 1) * 512],
                            part_sb[(mt, oh)][:], pps[:])
                        (nc.sync if (mt * 2 + oh) % 2 == 0
                         else nc.scalar).dma_start(
                            outD[mt * 128:(mt + 1) * 128,
                                 oh * 512:(oh + 1) * 512],
                            ost[:, oh * 512:(oh + 1) * 512],
                        )
                    yield ev

        # deferred phase-0 work, in first-consumed order
        fill_iters.append(("k1", g_qk0k(1)))
        fill_iters.append(("v1", g_v(1)))
        fill_iters.append(("k2", g_qk0k(2)))
        fill_iters.append(("v2", g_v(2)))
        fill_iters.append(("k3", g_qk0k(3)))
        fill_iters.append(("v3", g_v(3)))

        def chunk_mm(state, jj):
            h, ic, ypt, jmax = state
            g0 = max(512 * ic, 128 * jj)
            w = 512 * ic + 512 - g0
            nc.tensor.matmul(
                ypt[0:65, g0 - 512 * ic:g0 - 512 * ic + w],
                lhsT=v4[:, jj, h, 0:65],
                rhs=pt_slab[:, OFF[jj] + g0 - 128 * jj:
                            OFF[jj] + g0 - 128 * jj + w],
                start=(jj == 0),
                stop=(jj == jmax),
            )

        def chunk_start(p, hh, h, ic):
            """First two P@V matmuls for queries [512ic, 512ic+512) of head h.
            These touch only slab regions 0-1, so they may precede the next
            row's exps; the split keeps ACT fed while the long matmul run of
            chunk_finish executes."""
            if p == 0 and ic > 0:
                flush(f"v{ic}")
            ypt = yps.tile([128, 512], f32, tag="yps", name=f"y{h}_{ic}")
            jmax = 4 * ic + 3
            state = (h, ic, ypt, jmax)
            for jj in range(min(2, jmax + 1)):
                chunk_mm(state, jj)
                sprinkle()
            return state

        def chunk_finish(p, hh, state):
            h, ic, ypt, jmax = state
            for jj in range(2, jmax + 1):
                chunk_mm(state, jj)
                sprinkle()
            sums = nrm.tile([1, 512], f32, tag="sums", name=f"sm{h}_{ic}")
            nc.vector.tensor_copy(sums[:], ypt[64:65, :])
            rcp_row = nrm.tile([1, 512], f32, tag="rrow", name=f"rr{h}_{ic}")
            # NOTE: custom-DVE op; feeding it straight from PSUM yields
            # garbage on hardware (sim does not model this) - stage via SBUF.
            nc.vector.reciprocal_approx_fast(rcp_row[:], sums[:])
            rcp = nrm.tile([64, 512], f32, tag="rcp", name=f"rc{h}_{ic}")
            nc.gpsimd.partition_broadcast(rcp[:], rcp_row[:])
            hs = slice(hh * 64, hh * 64 + 64)
            nc.vector.tensor_mul(
                yT_tiles[p][hs, ic * 512:(ic + 1) * 512], ypt[0:64, :], rcp[:])
            if p == PAIRS - 1:
                if hh == 0:
                    if ic > 0:
                        fill_iters.append((f"projA{ic}", g_projA(ic)))
                elif ic == 0:
                    fill_iters.append(("proj0", g_proj0()))
                else:
                    fill_iters.append((f"projB{ic}", g_projB(ic)))

        pending = None  # (p, hh, h, ic) chunk awaiting emission

        for p in range(PAIRS):
            if p < PAIRS - 1:
                fill_iters.append((f"qk{p + 1}", g_qk(p + 1)))
            # pairs 1-2 carry only ~72 filler thunks for ~160 sprinkle
            # points; halve the consumption rate so they last the pair.
            pace["n"] = 2 if p in (1, 2) else 1
            qT = qT_t[p]
            kTt = kT_t[p]
            for hh in range(2):
                h = 2 * p + hh
                hb = hh * 64
                for j in range(MT):
                    if p == 0 and j % 4 == 0 and j > 0:
                        flush(f"k{j // 4}")
                    W = T - 128 * j
                    base = OFF[j]
                    nsec = (W + 1023) // 1024
                    cstate = None
                    # section 0
                    sw0 = min(1024, W)
                    sp0 = sps.tile([128, 1024], f32, tag="sps",
                                   name=f"s{h}_{j}_0")
                    for half in range(0, sw0, 512):
                        hw_ = min(512, sw0 - half)
                        io = 128 * j + half
                        nc.tensor.matmul(
                            sp0[:, half:half + hw_],
                            lhsT=kTt[hb:hb + 64, j * 128:(j + 1) * 128],
                            rhs=qT[hb:hb + 64, io:io + hw_],
                            start=True,
                            stop=True,
                        )
                        sprinkle()
                    if pending is not None:
                        # must precede the exp that overwrites slab region 0
                        cstate = chunk_start(*pending)
                    nc.scalar.activation(
                        pt_slab[:, base:base + sw0], sp0[:, 0:sw0],
                        AF.Exp, scale=SC)
                    if nsec == 2:
                        sw1 = W - 1024
                        sp1 = sps.tile([128, 1024], f32, tag="sps",
                                       name=f"s{h}_{j}_1")
                        for half in range(0, sw1, 512):
                            hw_ = min(512, sw1 - half)
                            io = 128 * j + 1024 + half
                            nc.tensor.matmul(
                                sp1[:, half:half + hw_],
                                lhsT=kTt[hb:hb + 64, j * 128:(j + 1) * 128],
                                rhs=qT[hb:hb + 64, io:io + hw_],
                                start=True,
                                stop=True,
                            )
                            sprinkle()
                    if pending is not None:
                        chunk_finish(pending[0], pending[1], cstate)
                        pending = None
                    if nsec == 2:
                        nc.scalar.activation(
                            pt_slab[:, base + 1024:base + 1024 + sw1],
                            sp1[:, 0:sw1], AF.Exp, scale=SC)
                    # zero upper-triangular part of the diagonal block
                    nc.vector.tensor_mul(
                        pt_slab[:, base:base + 128],
                        pt_slab[:, base:base + 128], mask_tri[:])
                    if j % 4 == 3:
                        pending = (p, hh, h, j // 4)
            # pair p+1's qT/kT must be complete before its S sections
            pace["n"] = 1
            drain()

        if pending is not None:
            cstate = chunk_start(*pending)
            chunk_finish(pending[0], pending[1], cstate)
            pending = None
        drain()

    nc.compile()
    return nc


def _get_nc():
    if "nc" not in _CACHE:
        _CACHE["nc"] = _build_nc()
    return _CACHE["nc"]


def make_in_maps(x, Wq, bq, Wk, bk, Wv, bv, Wp, bp):
    import ml_dtypes

    bf = ml_dtypes.bfloat16
    x = np.asarray(x, np.float32)
    Wq = np.asarray(Wq, np.float32).astype(bf)
    Wk = np.asarray(Wk, np.float32).astype(bf)
    Wv = np.asarray(Wv, np.float32).astype(bf)
    Wp = np.asarray(Wp, np.float32).astype(bf)
    bq = np.asarray(bq, np.float32)
    bk = np.asarray(bk, np.float32)
    bv = np.asarray(bv, np.float32)
    in_maps = []
    for c in range(NCORES):
        b, hg = divmod(c, 2)
        sl = slice(hg * 512, (hg + 1) * 512)
        in_maps.append({
            "xT": np.ascontiguousarray(x[b].T.astype(bf)),
            "wq": np.ascontiguousarray(Wq[:, sl]),
            "wk": np.ascontiguousarray(Wk[:, sl]),
            "wv": np.ascontiguousarray(Wv[:, sl]),
            "wp": np.ascontiguousarray(Wp[sl, :]),
            "bq": np.ascontiguousarray(bq[sl]),
            "bk": np.ascontiguousarray(bk[sl]),
            "bv": np.ascontiguousarray(bv[sl]),
        })
    return in_maps


def combine(results, bp):
    bp = np.asarray(bp, np.float32)
    out = np.empty((B, T, C), np.float32)
    for b in range(B):
        out[b] = (np.asarray(results[2 * b]["out"], np.float32)
                  + np.asarray(results[2 * b + 1]["out"], np.float32) + bp)
    return out


def kernel(x, Wq, bq, Wk, bk, Wv, bv, Wp, bp):
    from concourse import bass_utils

    nc = _get_nc()
    in_maps = make_in_maps(x, Wq, bq, Wk, bk, Wv, bv, Wp, bp)
    res = bass_utils.run_bass_kernel_spmd(nc, in_maps, core_ids=list(range(NCORES)))
    return combine(res.results, bp)
